# revision 6
# baseline (speedup 1.0000x reference)
"""CFNet interaction block on 8 TRN2 NeuronCores (Bass/Tile).

Sharding: core c owns atoms [2500c, 2500(c+1)). seg_i/seg_j sorted => each
core's interactions and triples are contiguous ranges; all index prep is done
host-side in numpy, so no device collectives or gathers are needed.

Both segment-sums are expressed as one-hot matmuls on the tensor engine:
interactions are packed into 128-slot blocks (<=256 triples each, 2 triple
tiles per block); K2=16 slot-tiles form an atom window (<=128 atoms) for the
atom-dim segment sum.

ssp(z) = softplus(z) - ln2 is computed exactly in 2 ACT passes (no softplus
table in this toolchain): u = Exp(z); ssp = Ln(0.5*u + 0.5). Both live in the
natural_log_exp_and_others table set (single table load). The shifted form
includes the -ln2, so no downstream corrections are needed.

Per-core pipeline (feature-major <-> item-major alternating, transpose-free):
  mm1 (W1 stationary, N=512)          -> z1[f2, t] PSUM
  Exp(z1 + b1) -> u1; Ln(.5u1+.5)     -> h[f2, t] bf16 (exact ssp)
  mm2 (h-tile stationary)             -> z2[t, f3] PSUM  (+rank-1 b2 matmul
                                         only when b2 != 0)
  Exp(z2) -> u2; Ln(.5u2+.5)          -> w[t, f3] bf16 (exact ssp)
  seg1 one-hot matmul                 -> w_ij[e', f] PSUM
  Win matmul (x_j pre-gathered)       -> f_j[e', f] PSUM
  DVE: wf = w_ij * f_j (bf16)
  seg2 one-hot matmul (16 acc)        -> conv[a', f] PSUM
  fp32 atom stage: PE transpose, Wout, exact ssp(+bout), Wd, +bd; y = v + x_a
"""
import sys

sys.path.insert(0, "/opt/trn_rl_repo")

import numpy as np
import ml_dtypes

LN2 = float(np.log(2.0))
N_ATOMS = 20000
DIM = 128
N_CORES = 8
ATOMS_PER_CORE = N_ATOMS // N_CORES
K2 = 16              # slot-tiles per atom window
BLK_TRIPLES = 256    # triple slots per 128-slot block

bf16 = ml_dtypes.bfloat16


# --------------------------------------------------------------------------
# Host-side packing
# --------------------------------------------------------------------------

def _pack_core(seg_i, seg_j, a0, a1):
    e0, e1 = np.searchsorted(seg_i, [a0, a1])
    t0, t1 = np.searchsorted(seg_j, [e0, e1])
    ne = np.diff(np.searchsorted(seg_j[t0:t1], np.arange(e0, e1 + 1))).astype(np.int64)
    atom_of_e = (seg_i[e0:e1] - a0).astype(np.int64)
    e_start_of_atom = np.searchsorted(seg_i[e0:e1] - a0, np.arange(a1 - a0 + 1))

    slot_e = []
    win_atoms = [[]]
    awin = []
    cur_slot = 0
    cur_trip = 0
    blk_in_win = 0
    atoms_in_win = 0
    cur_win = 0

    def close_block():
        nonlocal cur_slot, cur_trip, blk_in_win
        slot_e.extend([-1] * (128 - cur_slot))
        cur_slot = 0
        cur_trip = 0
        blk_in_win += 1

    def close_window():
        nonlocal blk_in_win, atoms_in_win, cur_win
        if cur_slot > 0 or cur_trip > 0:
            close_block()
        while blk_in_win < K2:
            slot_e.extend([-1] * 128)
            blk_in_win += 1
        blk_in_win = 0
        atoms_in_win = 0
        cur_win += 1
        win_atoms.append([])

    for a in range(a1 - a0):
        es, ee = int(e_start_of_atom[a]), int(e_start_of_atom[a + 1])
        if atoms_in_win >= 128:
            close_window()
        s_slot, s_trip, s_blk = cur_slot, cur_trip, blk_in_win
        ok = True
        for e in range(es, ee):
            t = int(ne[e])
            assert t <= BLK_TRIPLES
            if s_slot >= 128 or s_trip + t > BLK_TRIPLES:
                s_blk += 1
                s_slot = 0
                s_trip = 0
                if s_blk >= K2:
                    ok = False
                    break
            s_slot += 1
            s_trip += t
        if not ok:
            close_window()
        for e in range(es, ee):
            t = int(ne[e])
            if cur_slot >= 128 or cur_trip + t > BLK_TRIPLES:
                close_block()
                assert blk_in_win < K2
            slot_e.append(e + e0)
            cur_slot += 1
            cur_trip += t
        win_atoms[cur_win].append(a)
        awin.append((cur_win, len(win_atoms[cur_win]) - 1))
        atoms_in_win += 1
    close_window()
    win_atoms.pop()

    slot_e = np.asarray(slot_e, dtype=np.int64)
    return dict(e0=e0, e1=e1, ne=ne, atom_of_e=atom_of_e,
                slot_e=slot_e, awin=np.asarray(awin).reshape(-1, 2),
                n_win=len(win_atoms))


def _build_core_arrays(core, x, dijk, idx_j, seg_j, pk, B1, NW):
    ES = B1 * 128
    T1S = B1 * 256
    A_SLOTS = NW * 128
    e0 = pk["e0"]
    ne = pk["ne"]
    slot_e = pk["slot_e"]
    slot_e_pad = np.full(ES, -1, dtype=np.int64)
    slot_e_pad[:len(slot_e)] = slot_e
    valid = slot_e_pad >= 0

    tstart = np.searchsorted(seg_j, np.arange(pk["e0"], pk["e1"]))
    trip_src = np.full(T1S, -1, dtype=np.int64)
    m1_rows = np.full(T1S, -1, dtype=np.int64)
    sb = slot_e_pad.reshape(B1, 128)
    for b in range(B1):
        pos = 0
        base = b * 256
        for sp in range(128):
            e = sb[b, sp]
            if e < 0:
                continue
            k = int(ne[e - e0])
            if k == 0:
                continue
            ts_ = int(tstart[e - e0])
            trip_src[base + pos: base + pos + k] = np.arange(ts_, ts_ + k)
            m1_rows[base + pos: base + pos + k] = sp
            pos += k
        assert pos <= 256

    dT = np.zeros((DIM, T1S), dtype=bf16)
    m = trip_src >= 0
    dT[:, m] = dijk[trip_src[m]].T.astype(bf16)

    m1 = np.zeros((128, T1S), dtype=bf16)
    tj = np.arange(T1S) // 128
    tr = np.arange(T1S) % 128
    mm = m1_rows >= 0
    m1[tr[mm], tj[mm] * 128 + m1_rows[mm]] = 1.0

    xT = np.zeros((DIM, ES), dtype=bf16)
    xT[:, valid] = x[idx_j[slot_e_pad[valid]]].T.astype(bf16)

    m2 = np.zeros((128, ES), dtype=bf16)
    aw = pk["awin"]
    atom_loc = pk["atom_of_e"]
    sv = np.nonzero(valid)[0]
    e_loc = slot_e_pad[sv] - e0
    apos = aw[atom_loc[e_loc], 1]
    m2[sv % 128, (sv // 128) * 128 + apos] = 1.0

    xaT = np.zeros((DIM, A_SLOTS), dtype=np.float32)
    aslot_of_atom = aw[:, 0] * 128 + aw[:, 1]
    a0 = core * ATOMS_PER_CORE
    xaT[:, aslot_of_atom] = x[a0: a0 + ATOMS_PER_CORE].T
    return dict(dT=dT, m1=m1, xT=xT, m2=m2, xaT=xaT,
                aslot_of_atom=aslot_of_atom)


def _prepare(inputs):
    x = np.asarray(inputs["x"], dtype=np.float32)
    dijk = np.asarray(inputs["dijk"], dtype=np.float32)
    idx_j = np.asarray(inputs["idx_j"]).astype(np.int64)
    seg_i = np.asarray(inputs["seg_i"]).astype(np.int64)
    seg_j = np.asarray(inputs["seg_j"]).astype(np.int64)
    assert int(inputs["seg_i_sum"]) == N_ATOMS

    packs = [_pack_core(seg_i, seg_j, c * ATOMS_PER_CORE, (c + 1) * ATOMS_PER_CORE)
             for c in range(N_CORES)]
    NW = max(p["n_win"] for p in packs)
    B1 = NW * K2
    cores = [_build_core_arrays(c, x, dijk, idx_j, seg_j, packs[c], B1, NW)
             for c in range(N_CORES)]

    b2 = np.asarray(inputs["b2"], np.float32)
    use_b2 = bool(np.any(b2 != 0.0))
    b2hi = b2.astype(bf16)
    b2lo = (b2 - b2hi.astype(np.float32)).astype(bf16)
    consts = dict(
        W1=np.asarray(inputs["W1"], np.float32).astype(bf16),
        W2=np.asarray(inputs["W2"], np.float32).astype(bf16),
        Win=np.asarray(inputs["Win"], np.float32).astype(bf16),
        Wout=np.asarray(inputs["Wout"], np.float32),
        Wd=np.asarray(inputs["Wd"], np.float32),
        b1=np.asarray(inputs["b1"], np.float32).reshape(128, 1),
        bout=np.asarray(inputs["bout"], np.float32).reshape(128, 1),
        bd=np.asarray(inputs["bd"], np.float32).reshape(128, 1),
        bhilo=np.stack([b2hi, b2lo]).astype(bf16),
    )
    return dict(B1=B1, NW=NW, cores=cores, consts=consts, use_b2=use_b2)


# --------------------------------------------------------------------------
# Device program
# --------------------------------------------------------------------------

_PROGRAM_CACHE = {}


def _build_program(B1, NW, use_b2):
    import concourse.mybir as mybir
    import concourse.tile as tile
    from concourse import bacc
    from concourse.masks import make_identity

    dt = mybir.dt
    AF = mybir.ActivationFunctionType
    ES = B1 * 128
    T1S = B1 * 256
    A_SLOTS = NW * 128
    NI = B1 // 2          # pipeline iterations: 512 triple-slots (2 blocks)
    NS = NI // 4          # 2048-col slabs

    nc = bacc.Bacc("TRN2", target_bir_lowering=False, debug=False)

    def din(name, shape, dtype):
        return nc.dram_tensor(name, shape, dtype, kind="ExternalInput").ap()

    dT_d = din("dT", [DIM, T1S], dt.bfloat16)
    m1_d = din("m1", [128, T1S], dt.bfloat16)
    xT_d = din("xT", [DIM, ES], dt.bfloat16)
    m2_d = din("m2", [128, ES], dt.bfloat16)
    xaT_d = din("xaT", [DIM, A_SLOTS], dt.float32)
    W1_d = din("W1", [128, 128], dt.bfloat16)
    W2_d = din("W2", [128, 128], dt.bfloat16)
    Win_d = din("Win", [128, 128], dt.bfloat16)
    Wout_d = din("Wout", [128, 128], dt.float32)
    Wd_d = din("Wd", [128, 128], dt.float32)
    b1_d = din("b1", [128, 1], dt.float32)
    bout_d = din("bout", [128, 1], dt.float32)
    bd_d = din("bd", [128, 1], dt.float32)
    bhilo_d = din("bhilo", [2, 128], dt.bfloat16)

    yT_d = nc.dram_tensor("yT", [DIM, A_SLOTS], dt.float32, kind="ExternalOutput").ap()
    vT_d = nc.dram_tensor("vT", [DIM, A_SLOTS], dt.float32, kind="ExternalOutput").ap()

    with tile.TileContext(nc) as tc:
        with (
            tc.tile_pool(name="const", bufs=1) as cpool,
            tc.tile_pool(name="dTp", bufs=3) as dT_pool,
            tc.tile_pool(name="m1p", bufs=14) as m1_pool,
            tc.tile_pool(name="xTp", bufs=8) as xT_pool,
            tc.tile_pool(name="m2p", bufs=14) as m2_pool,
            tc.tile_pool(name="u1p", bufs=2) as u1_pool,
            tc.tile_pool(name="u2p", bufs=2) as u2_pool,
            tc.tile_pool(name="hp", bufs=2) as h_pool,
            tc.tile_pool(name="wp", bufs=2) as w_pool,
            tc.tile_pool(name="fjp", bufs=4) as fj_pool,
            tc.tile_pool(name="wfp", bufs=12) as wf_pool,
            tc.tile_pool(name="atoms", bufs=2) as at_pool,
            tc.tile_pool(name="vy", bufs=3) as vy_pool,
            tc.tile_pool(name="z1p", bufs=2, space="PSUM") as z1_pool,
            tc.tile_pool(name="z2p", bufs=2, space="PSUM") as z2_pool,
            tc.tile_pool(name="sfp", bufs=2, space="PSUM") as sf_pool,
            tc.tile_pool(name="atp", bufs=2, space="PSUM") as atp_pool,
        ):
            # ---- constants
            def cload(name, shape, dtype, src):
                t = cpool.tile(shape, dtype, tag=name, name=name)
                nc.sync.dma_start(out=t[:], in_=src[:])
                return t

            W1s = cload("W1s", [128, 128], dt.bfloat16, W1_d)
            W2s = cload("W2s", [128, 128], dt.bfloat16, W2_d)
            Wins = cload("Wins", [128, 128], dt.bfloat16, Win_d)
            Wouts = cload("Wouts", [128, 128], dt.float32, Wout_d)
            Wds = cload("Wds", [128, 128], dt.float32, Wd_d)
            b1s = cload("b1s", [128, 1], dt.float32, b1_d)
            bouts = cload("bouts", [128, 1], dt.float32, bout_d)
            bds = cload("bds", [128, 1], dt.float32, bd_d)
            bhilos = cload("bhilos", [2, 128], dt.bfloat16, bhilo_d)
            xaTs = cload("xaTs", [128, A_SLOTS], dt.float32, xaT_d)
            ones2 = cpool.tile([2, 128], dt.bfloat16, tag="ones2")
            nc.vector.memset(ones2[:], 1.0)
            halfs = cpool.tile([128, 1], dt.float32, tag="halfs")
            nc.vector.memset(halfs[:], 0.5)
            ident = cpool.tile([128, 128], dt.float32, tag="ident")
            make_identity(nc, ident[:])

            st = {}        # i2 -> per-iteration tiles
            slabs = {}     # k4 -> slab tiles (u1/h/u2/w)
            st_m2 = {}     # m2 chunk (4 blocks) -> tile
            wf_ring = []   # (i2, tile[128,256]) pairs in block order

            def ld(i2):
                if not (0 <= i2 < NI):
                    return
                s = {}
                s["dT"] = dT_pool.tile([128, 512], dt.bfloat16, tag="dT", name="dT")
                nc.sync.dma_start(out=s["dT"][:], in_=dT_d[:, i2 * 512:(i2 + 1) * 512])
                s["m1"] = m1_pool.tile([128, 512], dt.bfloat16, tag="m1", name="m1")
                nc.sync.dma_start(out=s["m1"][:], in_=m1_d[:, i2 * 512:(i2 + 1) * 512])
                if i2 % 2 == 0:
                    s["xT"] = xT_pool.tile([128, 512], dt.bfloat16, tag="xT", name="xT")
                    nc.sync.dma_start(out=s["xT"][:], in_=xT_d[:, i2 * 256:(i2 + 2) * 256])
                    m2t = m2_pool.tile([128, 512], dt.bfloat16, tag="m2")
                    nc.sync.dma_start(out=m2t[:], in_=m2_d[:, i2 * 256:(i2 + 2) * 256])
                    st_m2[i2 // 2] = m2t
                else:
                    s["xT"] = st[i2 - 1]["xT"]
                st[i2] = s

            def s_mm1(i2):
                if not (0 <= i2 < NI):
                    return
                k4, r4 = divmod(i2, 4)
                if r4 == 0:
                    slabs[k4] = {"u1": u1_pool.tile([128, 2048], dt.float32, tag="u1", name="u1")}
                z1 = z1_pool.tile([128, 512], dt.float32, tag="z1")
                nc.tensor.matmul(z1[:], lhsT=W1s[:], rhs=st[i2]["dT"][:],
                                 start=True, stop=True)
                nc.scalar.activation(slabs[k4]["u1"][:, r4 * 512:(r4 + 1) * 512],
                                     z1[:], AF.Exp, bias=b1s[:], scale=1.0)

            def s_ln1(k4):
                if not (0 <= k4 < NS):
                    return
                h = h_pool.tile([128, 2048], dt.bfloat16, tag="h")
                nc.scalar.activation(h[:], slabs[k4]["u1"][:], AF.Ln,
                                     bias=halfs[:], scale=0.5)
                slabs[k4]["h"] = h

            def s_mm2(i2):
                if not (0 <= i2 < NI):
                    return
                k4, r4 = divmod(i2, 4)
                if r4 == 0:
                    slabs[k4]["u2"] = u2_pool.tile([128, 2048], dt.float32, tag="u2", name="u2")
                h = slabs[k4]["h"]
                z2 = z2_pool.tile([128, 512], dt.float32, tag="z2")
                for k in range(4):
                    sl = z2[:, k * 128:(k + 1) * 128]
                    hsl = h[:, (r4 * 4 + k) * 128:(r4 * 4 + k + 1) * 128]
                    if use_b2:
                        nc.tensor.matmul(sl, lhsT=ones2[:], rhs=bhilos[:],
                                         start=True, stop=False)
                        nc.tensor.matmul(sl, lhsT=hsl, rhs=W2s[:],
                                         start=False, stop=True)
                    else:
                        nc.tensor.matmul(sl, lhsT=hsl, rhs=W2s[:],
                                         start=True, stop=True)
                nc.scalar.activation(slabs[k4]["u2"][:, r4 * 512:(r4 + 1) * 512],
                                     z2[:], AF.Exp)

            def s_ln2(k4):
                if not (0 <= k4 < NS):
                    return
                w = w_pool.tile([128, 2048], dt.bfloat16, tag="w")
                nc.scalar.activation(w[:], slabs[k4]["u2"][:], AF.Ln,
                                     bias=halfs[:], scale=0.5)
                slabs[k4]["w"] = w

            def s_seg(i2):
                if not (0 <= i2 < NI):
                    return
                k4, r4 = divmod(i2, 4)
                s = st[i2]
                w = slabs[k4]["w"]
                sf = sf_pool.tile([128, 512], dt.float32, tag="sf")
                for bb in range(2):
                    dst = sf[:, bb * 128:(bb + 1) * 128]
                    for k in range(2):
                        j = 2 * bb + k
                        nc.tensor.matmul(
                            dst,
                            lhsT=s["m1"][:, j * 128:(j + 1) * 128],
                            rhs=w[:, (r4 * 4 + j) * 128:(r4 * 4 + j + 1) * 128],
                            start=(k == 0), stop=(k == 1))
                    xcol = (i2 % 2) * 2 + bb
                    nc.tensor.matmul(sf[:, 256 + bb * 128:256 + (bb + 1) * 128],
                                     lhsT=s["xT"][:, xcol * 128:(xcol + 1) * 128],
                                     rhs=Wins[:], start=True, stop=True)
                fj = fj_pool.tile([128, 256], dt.bfloat16, tag="fj")
                nc.vector.tensor_copy(fj[:], sf[:, 256:512])
                wf = wf_pool.tile([128, 256], dt.bfloat16, tag="wf")
                nc.vector.tensor_mul(wf[:], sf[:, 0:256], fj[:])
                wf_ring.append((i2, wf))
                del st[i2]["dT"]

            def s_atom(wi):
                if not (0 <= wi < NW):
                    return
                at = atp_pool.tile([128, 512], dt.float32, tag="at")
                conv = at[:, 0:128]
                for k in range(K2):
                    b = wi * K2 + k
                    i2w, wf = wf_ring[0]
                    assert i2w == b // 2
                    m2t = st_m2[b // 4]
                    nc.tensor.matmul(conv,
                                     lhsT=m2t[:, (b % 4) * 128:(b % 4 + 1) * 128],
                                     rhs=wf[:, (b % 2) * 128:(b % 2 + 1) * 128],
                                     start=(k == 0), stop=(k == K2 - 1))
                    if b % 2 == 1:
                        wf_ring.pop(0)
                conv_s = at_pool.tile([128, 128], dt.float32, tag="conv_s")
                nc.vector.tensor_copy(conv_s[:], conv)
                nc.tensor.transpose(at[:, 128:256], conv_s[:], ident[:])
                cT = at_pool.tile([128, 128], dt.float32, tag="cT")
                nc.vector.tensor_copy(cT[:], at[:, 128:256])
                nc.tensor.matmul(at[:, 256:384], lhsT=Wouts[:], rhs=cT[:],
                                 start=True, stop=True)
                u3 = at_pool.tile([128, 128], dt.float32, tag="u3")
                nc.scalar.activation(u3[:], at[:, 256:384], AF.Exp, bias=bouts[:])
                c3 = at_pool.tile([128, 128], dt.float32, tag="c3")
                nc.scalar.activation(c3[:], u3[:], AF.Ln, bias=halfs[:], scale=0.5)
                nc.tensor.matmul(at[:, 384:512], lhsT=Wds[:], rhs=c3[:],
                                 start=True, stop=True)
                vt = vy_pool.tile([128, 128], dt.float32, tag="vt")
                nc.vector.tensor_scalar_add(vt[:], at[:, 384:512], bds[:])
                yt = vy_pool.tile([128, 128], dt.float32, tag="yt")
                nc.vector.tensor_add(yt[:], vt[:], xaTs[:, wi * 128:(wi + 1) * 128])
                nc.sync.dma_start(out=vT_d[:, wi * 128:(wi + 1) * 128], in_=vt[:])
                nc.sync.dma_start(out=yT_d[:, wi * 128:(wi + 1) * 128], in_=yt[:])

            # pipeline: ld(n) | mm1(n-2) | ln1 | mm2(n-6) | ln2 | seg(n-10) | atom
            for n in range(NI + 13):
                ld(n)
                s_mm1(n - 2)
                if (n - 2) % 4 == 3:
                    s_ln1((n - 2) // 4)
                s_mm2(n - 6)
                if (n - 6) % 4 == 3:
                    s_ln2((n - 6) // 4)
                s_seg(n - 10)
                i2s = n - 10
                if i2s >= 0 and i2s % 8 == 7:
                    s_atom(i2s // 8)
    nc.compile()
    return nc


def _get_program(B1, NW, use_b2):
    key = (B1, NW, use_b2)
    if key not in _PROGRAM_CACHE:
        _PROGRAM_CACHE[key] = _build_program(B1, NW, use_b2)
    return _PROGRAM_CACHE[key]


def kernel(**inputs):
    from concourse.bass_utils import run_bass_kernel_spmd

    prep = _prepare(inputs)
    nc = _get_program(prep["B1"], prep["NW"], prep["use_b2"])
    consts = prep["consts"]
    in_maps = []
    for c in range(N_CORES):
        cc = prep["cores"][c]
        in_maps.append(dict(
            dT=np.ascontiguousarray(cc["dT"]),
            m1=np.ascontiguousarray(cc["m1"]),
            xT=np.ascontiguousarray(cc["xT"]),
            m2=np.ascontiguousarray(cc["m2"]),
            xaT=np.ascontiguousarray(cc["xaT"]),
            W1=consts["W1"], W2=consts["W2"], Win=consts["Win"],
            Wout=consts["Wout"], Wd=consts["Wd"],
            b1=consts["b1"], bout=consts["bout"], bd=consts["bd"],
            bhilo=consts["bhilo"],
        ))
    res = run_bass_kernel_spmd(nc, in_maps, list(range(N_CORES)))
    y = np.zeros((N_ATOMS, DIM), np.float32)
    v = np.zeros((N_ATOMS, DIM), np.float32)
    for c in range(N_CORES):
        sl = prep["cores"][c]["aslot_of_atom"]
        a0 = c * ATOMS_PER_CORE
        y[a0:a0 + ATOMS_PER_CORE] = res.results[c]["yT"][:, sl].T
        v[a0:a0 + ATOMS_PER_CORE] = res.results[c]["vT"][:, sl].T
    return y, v


# revision 11
# speedup vs baseline: 1.1650x; 1.1650x over previous
"""CFNet interaction block on 8 TRN2 NeuronCores (Bass/Tile).

Sharding: core c owns atoms [2500c, 2500(c+1)). seg_i/seg_j sorted => each
core's interactions and triples are contiguous ranges; all index prep is done
host-side in numpy, so no device collectives or gathers are needed.

Both segment-sums are expressed as one-hot matmuls on the tensor engine:
interactions are packed into 128-slot blocks (<=256 triples each, 2 triple
tiles per block); K2=16 slot-tiles form an atom window (<=128 atoms) for the
atom-dim segment sum.

ssp(z) = softplus(z) - ln2 is computed exactly in 2 ACT passes (no softplus
table in this toolchain): u = Exp(z); ssp = Ln(0.5*u + 0.5). Both live in the
natural_log_exp_and_others table set (single table load). The shifted form
includes the -ln2, so no downstream corrections are needed.

Per-core pipeline (feature-major <-> item-major alternating, transpose-free):
  mm1 (W1 stationary, N=512)          -> z1[f2, t] PSUM
  Exp(z1 + b1) -> u1; Ln(.5u1+.5)     -> h[f2, t] bf16 (exact ssp)
  mm2 (h-tile stationary)             -> z2[t, f3] PSUM  (+rank-1 b2 matmul
                                         only when b2 != 0)
  Exp(z2) -> u2; Ln(.5u2+.5)          -> w[t, f3] bf16 (exact ssp)
  seg1 one-hot matmul                 -> w_ij[e', f] PSUM
  Win matmul (x_j pre-gathered)       -> f_j[e', f] PSUM
  DVE: wf = w_ij * f_j (bf16)
  seg2 one-hot matmul (16 acc)        -> conv[a', f] PSUM
  fp32 atom stage: PE transpose, Wout, exact ssp(+bout), Wd, +bd; y = v + x_a
"""
import sys

sys.path.insert(0, "/opt/trn_rl_repo")

import numpy as np
import ml_dtypes

LN2 = float(np.log(2.0))
N_ATOMS = 20000
DIM = 128
N_CORES = 8
ATOMS_PER_CORE = N_ATOMS // N_CORES
K2 = 16              # slot-tiles per atom window
BLK_TRIPLES = 256    # triple slots per 128-slot block

bf16 = ml_dtypes.bfloat16


# --------------------------------------------------------------------------
# Host-side packing
# --------------------------------------------------------------------------

def _pack_core(seg_i, seg_j, a0, a1):
    e0, e1 = np.searchsorted(seg_i, [a0, a1])
    t0, t1 = np.searchsorted(seg_j, [e0, e1])
    ne = np.diff(np.searchsorted(seg_j[t0:t1], np.arange(e0, e1 + 1))).astype(np.int64)
    atom_of_e = (seg_i[e0:e1] - a0).astype(np.int64)
    e_start_of_atom = np.searchsorted(seg_i[e0:e1] - a0, np.arange(a1 - a0 + 1))

    slot_e = []
    win_atoms = [[]]
    awin = []
    cur_slot = 0
    cur_trip = 0
    blk_in_win = 0
    atoms_in_win = 0
    cur_win = 0

    def close_block():
        nonlocal cur_slot, cur_trip, blk_in_win
        slot_e.extend([-1] * (128 - cur_slot))
        cur_slot = 0
        cur_trip = 0
        blk_in_win += 1

    def close_window():
        nonlocal blk_in_win, atoms_in_win, cur_win
        if cur_slot > 0 or cur_trip > 0:
            close_block()
        while blk_in_win < K2:
            slot_e.extend([-1] * 128)
            blk_in_win += 1
        blk_in_win = 0
        atoms_in_win = 0
        cur_win += 1
        win_atoms.append([])

    for a in range(a1 - a0):
        es, ee = int(e_start_of_atom[a]), int(e_start_of_atom[a + 1])
        if atoms_in_win >= 128:
            close_window()
        s_slot, s_trip, s_blk = cur_slot, cur_trip, blk_in_win
        ok = True
        for e in range(es, ee):
            t = int(ne[e])
            assert t <= BLK_TRIPLES
            if s_slot >= 128 or s_trip + t > BLK_TRIPLES:
                s_blk += 1
                s_slot = 0
                s_trip = 0
                if s_blk >= K2:
                    ok = False
                    break
            s_slot += 1
            s_trip += t
        if not ok:
            close_window()
        for e in range(es, ee):
            t = int(ne[e])
            if cur_slot >= 128 or cur_trip + t > BLK_TRIPLES:
                close_block()
                assert blk_in_win < K2
            slot_e.append(e + e0)
            cur_slot += 1
            cur_trip += t
        win_atoms[cur_win].append(a)
        awin.append((cur_win, len(win_atoms[cur_win]) - 1))
        atoms_in_win += 1
    close_window()
    win_atoms.pop()

    slot_e = np.asarray(slot_e, dtype=np.int64)
    return dict(e0=e0, e1=e1, ne=ne, atom_of_e=atom_of_e,
                slot_e=slot_e, awin=np.asarray(awin).reshape(-1, 2),
                n_win=len(win_atoms))


def _build_core_arrays(core, x, dijk, idx_j, seg_j, pk, B1, NW):
    ES = B1 * 128
    T1S = B1 * 256
    A_SLOTS = NW * 128
    e0 = pk["e0"]
    ne = pk["ne"]
    slot_e = pk["slot_e"]
    slot_e_pad = np.full(ES, -1, dtype=np.int64)
    slot_e_pad[:len(slot_e)] = slot_e
    valid = slot_e_pad >= 0

    tstart = np.searchsorted(seg_j, np.arange(pk["e0"], pk["e1"]))
    trip_src = np.full(T1S, -1, dtype=np.int64)
    m1_rows = np.full(T1S, -1, dtype=np.int64)
    sb = slot_e_pad.reshape(B1, 128)
    for b in range(B1):
        pos = 0
        base = b * 256
        for sp in range(128):
            e = sb[b, sp]
            if e < 0:
                continue
            k = int(ne[e - e0])
            if k == 0:
                continue
            ts_ = int(tstart[e - e0])
            trip_src[base + pos: base + pos + k] = np.arange(ts_, ts_ + k)
            m1_rows[base + pos: base + pos + k] = sp
            pos += k
        assert pos <= 256

    dT = np.zeros((DIM, T1S), dtype=bf16)
    m = trip_src >= 0
    dT[:, m] = dijk[trip_src[m]].T.astype(bf16)

    m1 = np.zeros((128, T1S), dtype=bf16)
    tj = np.arange(T1S) // 128
    tr = np.arange(T1S) % 128
    mm = m1_rows >= 0
    m1[tr[mm], tj[mm] * 128 + m1_rows[mm]] = 1.0

    xT = np.zeros((DIM, ES), dtype=bf16)
    xT[:, valid] = x[idx_j[slot_e_pad[valid]]].T.astype(bf16)

    m2 = np.zeros((128, ES), dtype=bf16)
    aw = pk["awin"]
    atom_loc = pk["atom_of_e"]
    sv = np.nonzero(valid)[0]
    e_loc = slot_e_pad[sv] - e0
    apos = aw[atom_loc[e_loc], 1]
    m2[sv % 128, (sv // 128) * 128 + apos] = 1.0

    xaT = np.zeros((DIM, A_SLOTS), dtype=np.float32)
    aslot_of_atom = aw[:, 0] * 128 + aw[:, 1]
    a0 = core * ATOMS_PER_CORE
    xaT[:, aslot_of_atom] = x[a0: a0 + ATOMS_PER_CORE].T
    return dict(dT=dT, m1=m1, xT=xT, m2=m2, xaT=xaT,
                aslot_of_atom=aslot_of_atom)


def _prepare(inputs):
    x = np.asarray(inputs["x"], dtype=np.float32)
    dijk = np.asarray(inputs["dijk"], dtype=np.float32)
    idx_j = np.asarray(inputs["idx_j"]).astype(np.int64)
    seg_i = np.asarray(inputs["seg_i"]).astype(np.int64)
    seg_j = np.asarray(inputs["seg_j"]).astype(np.int64)
    assert int(inputs["seg_i_sum"]) == N_ATOMS

    packs = [_pack_core(seg_i, seg_j, c * ATOMS_PER_CORE, (c + 1) * ATOMS_PER_CORE)
             for c in range(N_CORES)]
    NW = max(p["n_win"] for p in packs)
    B1 = NW * K2
    cores = [_build_core_arrays(c, x, dijk, idx_j, seg_j, packs[c], B1, NW)
             for c in range(N_CORES)]

    b2 = np.asarray(inputs["b2"], np.float32)
    use_b2 = bool(np.any(b2 != 0.0))
    b2hi = b2.astype(bf16)
    b2lo = (b2 - b2hi.astype(np.float32)).astype(bf16)
    consts = dict(
        W1=np.asarray(inputs["W1"], np.float32).astype(bf16),
        W2=np.asarray(inputs["W2"], np.float32).astype(bf16),
        Win=np.asarray(inputs["Win"], np.float32).astype(bf16),
        Wout=np.asarray(inputs["Wout"], np.float32),
        Wd=np.asarray(inputs["Wd"], np.float32),
        b1=np.asarray(inputs["b1"], np.float32).reshape(128, 1),
        bout=np.asarray(inputs["bout"], np.float32).reshape(128, 1),
        bd=np.asarray(inputs["bd"], np.float32).reshape(128, 1),
        bhilo=np.stack([b2hi, b2lo]).astype(bf16),
    )
    return dict(B1=B1, NW=NW, cores=cores, consts=consts, use_b2=use_b2)


# --------------------------------------------------------------------------
# Device program
# --------------------------------------------------------------------------

_PROGRAM_CACHE = {}


def _build_program(B1, NW, use_b2):
    import bass_rust as _bass_rust
    import concourse.mybir as mybir
    import concourse.tile as tile
    from concourse import bacc
    from concourse.hw_specs import get_activation_tables
    from concourse.masks import make_identity

    class _Bacc(bacc.Bacc):
        """Force every activation onto the single exp+ln table set so the
        Exp/Ln alternation never reloads ACT tables (index into the table
        list is the act_func_set_id, so other entries are emptied, not
        removed)."""

        def insert_act_table_loads(self):
            import concourse.mybir as mb
            has_activation = any(
                isinstance(i, mb.InstActivation)
                for b in self.main_func.blocks
                for i in b.instructions
            )
            if not has_activation:
                return
            tables = []
            for name, funcs in get_activation_tables(self.m.arch).items():
                if name == "natural_log_exp_and_others":
                    tables.append((name, funcs))
                else:
                    tables.append((name, type(funcs)()))
            _bass_rust.insert_act_table_loads(self, tables)

    dt = mybir.dt
    AF = mybir.ActivationFunctionType
    ES = B1 * 128
    T1S = B1 * 256
    A_SLOTS = NW * 128
    NI = B1 // 2          # pipeline iterations: 512 triple-slots (2 blocks)
    NS = NI // 4          # 2048-col slabs

    nc = _Bacc("TRN2", target_bir_lowering=False, debug=False)

    def din(name, shape, dtype):
        return nc.dram_tensor(name, shape, dtype, kind="ExternalInput").ap()

    dT_d = din("dT", [DIM, T1S], dt.bfloat16)
    m1_d = din("m1", [128, T1S], dt.bfloat16)
    xT_d = din("xT", [DIM, ES], dt.bfloat16)
    m2_d = din("m2", [128, ES], dt.bfloat16)
    xaT_d = din("xaT", [DIM, A_SLOTS], dt.float32)
    W1_d = din("W1", [128, 128], dt.bfloat16)
    W2_d = din("W2", [128, 128], dt.bfloat16)
    Win_d = din("Win", [128, 128], dt.bfloat16)
    Wout_d = din("Wout", [128, 128], dt.float32)
    Wd_d = din("Wd", [128, 128], dt.float32)
    b1_d = din("b1", [128, 1], dt.float32)
    bout_d = din("bout", [128, 1], dt.float32)
    bd_d = din("bd", [128, 1], dt.float32)
    bhilo_d = din("bhilo", [2, 128], dt.bfloat16)

    yT_d = nc.dram_tensor("yT", [DIM, A_SLOTS], dt.float32, kind="ExternalOutput").ap()
    vT_d = nc.dram_tensor("vT", [DIM, A_SLOTS], dt.float32, kind="ExternalOutput").ap()

    with tile.TileContext(nc) as tc:
        with (
            tc.tile_pool(name="const", bufs=1) as cpool,
            tc.tile_pool(name="dTp", bufs=3) as dT_pool,
            tc.tile_pool(name="m1p", bufs=7) as m1_pool,
            tc.tile_pool(name="xTp", bufs=7) as xT_pool,
            tc.tile_pool(name="m2p", bufs=8) as m2_pool,
            tc.tile_pool(name="u1p", bufs=2) as u1_pool,
            tc.tile_pool(name="u2p", bufs=2) as u2_pool,
            tc.tile_pool(name="hp", bufs=2) as h_pool,
            tc.tile_pool(name="wp", bufs=2) as w_pool,
            tc.tile_pool(name="fjp", bufs=4) as fj_pool,
            tc.tile_pool(name="wfp", bufs=12) as wf_pool,
            tc.tile_pool(name="atoms", bufs=2) as at_pool,
            tc.tile_pool(name="vy", bufs=3) as vy_pool,
            tc.tile_pool(name="zp", bufs=2, space="PSUM") as z_pool,
            tc.tile_pool(name="sfp", bufs=2, space="PSUM") as sf_pool,
            tc.tile_pool(name="atp", bufs=2, space="PSUM") as atp_pool,
        ):
            # ---- constants
            def cload(name, shape, dtype, src):
                t = cpool.tile(shape, dtype, tag=name, name=name)
                nc.sync.dma_start(out=t[:], in_=src[:])
                return t

            W1s = cload("W1s", [128, 128], dt.bfloat16, W1_d)
            W2s = cload("W2s", [128, 128], dt.bfloat16, W2_d)
            Wins = cload("Wins", [128, 128], dt.bfloat16, Win_d)
            Wouts = cload("Wouts", [128, 128], dt.float32, Wout_d)
            Wds = cload("Wds", [128, 128], dt.float32, Wd_d)
            b1s = cload("b1s", [128, 1], dt.float32, b1_d)
            bouts = cload("bouts", [128, 1], dt.float32, bout_d)
            bds = cload("bds", [128, 1], dt.float32, bd_d)
            bhilos = cload("bhilos", [2, 128], dt.bfloat16, bhilo_d)
            xaTs = cload("xaTs", [128, A_SLOTS], dt.float32, xaT_d)
            ones2 = cpool.tile([2, 128], dt.bfloat16, tag="ones2")
            nc.vector.memset(ones2[:], 1.0)
            halfs = cpool.tile([128, 1], dt.float32, tag="halfs")
            nc.vector.memset(halfs[:], 0.5)
            ident = cpool.tile([128, 128], dt.float32, tag="ident")
            make_identity(nc, ident[:])

            NP = B1 // 4   # pipeline unit p: 1024 triple cols / 512 slots
            st = {}        # slab q (2048 cols) -> loaded tiles
            slabs = {}     # q -> u1/h/u2/w slabs
            st_m2 = {}     # q -> m2 slab
            wf_ring = []   # (block-pair index, tile[128,256])

            def ld(q):
                if not (0 <= q < NS):
                    return
                s = {}
                s["dT"] = dT_pool.tile([128, 2048], dt.bfloat16, tag="dT", name="dT")
                nc.sync.dma_start(out=s["dT"][:], in_=dT_d[:, q * 2048:(q + 1) * 2048])
                s["m1"] = m1_pool.tile([128, 2048], dt.bfloat16, tag="m1", name="m1")
                nc.sync.dma_start(out=s["m1"][:], in_=m1_d[:, q * 2048:(q + 1) * 2048])
                s["xT"] = xT_pool.tile([128, 1024], dt.bfloat16, tag="xT", name="xT")
                nc.sync.dma_start(out=s["xT"][:], in_=xT_d[:, q * 1024:(q + 1) * 1024])
                m2t = m2_pool.tile([128, 1024], dt.bfloat16, tag="m2", name="m2")
                nc.sync.dma_start(out=m2t[:], in_=m2_d[:, q * 1024:(q + 1) * 1024])
                st_m2[q] = m2t
                st[q] = s

            def s_mm1(p):
                # 1024 triple cols: 2 matmuls (N=512) + one Exp @1024
                if not (0 <= p < NP):
                    return
                q, rp = divmod(p, 2)
                if rp == 0:
                    slabs[q] = {"u1": u1_pool.tile([128, 2048], dt.float32,
                                                   tag="u1", name="u1")}
                z = z_pool.tile([128, 1024], dt.float32, tag="z", name="z")
                for half in range(2):
                    nc.tensor.matmul(
                        z[:, half * 512:(half + 1) * 512], lhsT=W1s[:],
                        rhs=st[q]["dT"][:, (rp * 2 + half) * 512:
                                        (rp * 2 + half + 1) * 512],
                        start=True, stop=True)
                nc.scalar.activation(slabs[q]["u1"][:, rp * 1024:(rp + 1) * 1024],
                                     z[:], AF.Exp, bias=b1s[:], scale=1.0)

            def s_ln1(q):
                if not (0 <= q < NS):
                    return
                h = h_pool.tile([128, 2048], dt.bfloat16, tag="h")
                nc.scalar.activation(h[:], slabs[q]["u1"][:], AF.Ln,
                                     bias=halfs[:], scale=0.5)
                slabs[q]["h"] = h

            def s_mm2(p):
                if not (0 <= p < NP):
                    return
                q, rp = divmod(p, 2)
                if rp == 0:
                    slabs[q]["u2"] = u2_pool.tile([128, 2048], dt.float32,
                                                  tag="u2", name="u2")
                h = slabs[q]["h"]
                z = z_pool.tile([128, 1024], dt.float32, tag="z", name="z2")
                for k in range(8):
                    sl = z[:, k * 128:(k + 1) * 128]
                    hsl = h[:, (rp * 8 + k) * 128:(rp * 8 + k + 1) * 128]
                    if use_b2:
                        nc.tensor.matmul(sl, lhsT=ones2[:], rhs=bhilos[:],
                                         start=True, stop=False)
                        nc.tensor.matmul(sl, lhsT=hsl, rhs=W2s[:],
                                         start=False, stop=True)
                    else:
                        nc.tensor.matmul(sl, lhsT=hsl, rhs=W2s[:],
                                         start=True, stop=True)
                nc.scalar.activation(slabs[q]["u2"][:, rp * 1024:(rp + 1) * 1024],
                                     z[:], AF.Exp)

            def s_ln2(q):
                if not (0 <= q < NS):
                    return
                w = w_pool.tile([128, 2048], dt.bfloat16, tag="w")
                nc.scalar.activation(w[:], slabs[q]["u2"][:], AF.Ln,
                                     bias=halfs[:], scale=0.5)
                slabs[q]["w"] = w

            def s_seg(p):
                # two sf tiles (2 blocks each)
                if not (0 <= p < NP):
                    return
                q, rp = divmod(p, 2)
                s = st[q]
                w = slabs[q]["w"]
                for half in range(2):
                    sf = sf_pool.tile([128, 512], dt.float32, tag="sf", name="sf")
                    for bb in range(2):
                        dst = sf[:, bb * 128:(bb + 1) * 128]
                        for k in range(2):
                            j = rp * 8 + half * 4 + 2 * bb + k
                            nc.tensor.matmul(
                                dst,
                                lhsT=s["m1"][:, j * 128:(j + 1) * 128],
                                rhs=w[:, j * 128:(j + 1) * 128],
                                start=(k == 0), stop=(k == 1))
                        xcol = rp * 4 + half * 2 + bb
                        nc.tensor.matmul(
                            sf[:, 256 + bb * 128:256 + (bb + 1) * 128],
                            lhsT=s["xT"][:, xcol * 128:(xcol + 1) * 128],
                            rhs=Wins[:], start=True, stop=True)
                    fj = fj_pool.tile([128, 256], dt.bfloat16, tag="fj")
                    nc.vector.tensor_copy(fj[:], sf[:, 256:512])
                    wf = wf_pool.tile([128, 256], dt.bfloat16, tag="wf")
                    nc.vector.tensor_mul(wf[:], sf[:, 0:256], fj[:])
                    wf_ring.append((2 * p + half, wf))

            def s_atom(wi):
                if not (0 <= wi < NW):
                    return
                at = atp_pool.tile([128, 512], dt.float32, tag="at", name="at")
                conv = at[:, 0:128]
                for k in range(K2):
                    b = wi * K2 + k
                    i2w, wf = wf_ring[0]
                    assert i2w == b // 2
                    m2t = st_m2[b // 8]
                    nc.tensor.matmul(conv,
                                     lhsT=m2t[:, (b % 8) * 128:(b % 8 + 1) * 128],
                                     rhs=wf[:, (b % 2) * 128:(b % 2 + 1) * 128],
                                     start=(k == 0), stop=(k == K2 - 1))
                    if b % 2 == 1:
                        wf_ring.pop(0)
                conv_s = at_pool.tile([128, 128], dt.float32, tag="conv_s")
                nc.vector.tensor_copy(conv_s[:], conv)
                nc.tensor.transpose(at[:, 128:256], conv_s[:], ident[:])
                cT = at_pool.tile([128, 128], dt.float32, tag="cT")
                nc.vector.tensor_copy(cT[:], at[:, 128:256])
                nc.tensor.matmul(at[:, 256:384], lhsT=Wouts[:], rhs=cT[:],
                                 start=True, stop=True)
                u3 = at_pool.tile([128, 128], dt.float32, tag="u3")
                nc.scalar.activation(u3[:], at[:, 256:384], AF.Exp, bias=bouts[:])
                c3 = at_pool.tile([128, 128], dt.float32, tag="c3")
                nc.scalar.activation(c3[:], u3[:], AF.Ln, bias=halfs[:], scale=0.5)
                nc.tensor.matmul(at[:, 384:512], lhsT=Wds[:], rhs=c3[:],
                                 start=True, stop=True)
                vt = vy_pool.tile([128, 128], dt.float32, tag="vt")
                nc.vector.tensor_scalar_add(vt[:], at[:, 384:512], bds[:])
                yt = vy_pool.tile([128, 128], dt.float32, tag="yt")
                nc.vector.tensor_add(yt[:], vt[:], xaTs[:, wi * 128:(wi + 1) * 128])
                nc.sync.dma_start(out=vT_d[:, wi * 128:(wi + 1) * 128], in_=vt[:])
                nc.sync.dma_start(out=yT_d[:, wi * 128:(wi + 1) * 128], in_=yt[:])

            # pipeline: ld | mm1(n-2) | ln1 | mm2(n-4) | ln2 | seg(n-6) | atom
            for n in range(NP + 9):
                if n == 0:
                    ld(0)
                if n % 2 == 0:
                    ld(n // 2 + 1)
                s_mm1(n - 2)
                if (n - 2) % 2 == 1:
                    s_ln1((n - 2) // 2)
                s_mm2(n - 4)
                if (n - 4) % 2 == 1:
                    s_ln2((n - 4) // 2)
                s_seg(n - 6)
                ps = n - 6
                if ps >= 0 and ps % 4 == 3:
                    s_atom(ps // 4)
    nc.compile()
    return nc


def _get_program(B1, NW, use_b2):
    key = (B1, NW, use_b2)
    if key not in _PROGRAM_CACHE:
        _PROGRAM_CACHE[key] = _build_program(B1, NW, use_b2)
    return _PROGRAM_CACHE[key]


def kernel(**inputs):
    from concourse.bass_utils import run_bass_kernel_spmd

    prep = _prepare(inputs)
    nc = _get_program(prep["B1"], prep["NW"], prep["use_b2"])
    consts = prep["consts"]
    in_maps = []
    for c in range(N_CORES):
        cc = prep["cores"][c]
        in_maps.append(dict(
            dT=np.ascontiguousarray(cc["dT"]),
            m1=np.ascontiguousarray(cc["m1"]),
            xT=np.ascontiguousarray(cc["xT"]),
            m2=np.ascontiguousarray(cc["m2"]),
            xaT=np.ascontiguousarray(cc["xaT"]),
            W1=consts["W1"], W2=consts["W2"], Win=consts["Win"],
            Wout=consts["Wout"], Wd=consts["Wd"],
            b1=consts["b1"], bout=consts["bout"], bd=consts["bd"],
            bhilo=consts["bhilo"],
        ))
    res = run_bass_kernel_spmd(nc, in_maps, list(range(N_CORES)))
    y = np.zeros((N_ATOMS, DIM), np.float32)
    v = np.zeros((N_ATOMS, DIM), np.float32)
    for c in range(N_CORES):
        sl = prep["cores"][c]["aslot_of_atom"]
        a0 = c * ATOMS_PER_CORE
        y[a0:a0 + ATOMS_PER_CORE] = res.results[c]["yT"][:, sl].T
        v[a0:a0 + ATOMS_PER_CORE] = res.results[c]["vT"][:, sl].T
    return y, v


# revision 12
# speedup vs baseline: 239.2727x; 205.3915x over previous
"""CFNet interaction block on 8 TRN2 NeuronCores (Bass/Tile).

Sharding: core c owns atoms [2500c, 2500(c+1)). seg_i/seg_j sorted => each
core's interactions and triples are contiguous ranges; all index prep is done
host-side in numpy, so no device collectives or gathers are needed.

Both segment-sums are expressed as one-hot matmuls on the tensor engine:
interactions are packed into 128-slot blocks (<=256 triples each, 2 triple
tiles per block); K2=16 slot-tiles form an atom window (<=128 atoms) for the
atom-dim segment sum.

ssp(z) = softplus(z) - ln2 is computed exactly in 2 ACT passes (no softplus
table in this toolchain): u = Exp(z); ssp = Ln(0.5*u + 0.5). Both live in the
natural_log_exp_and_others table set (single table load). The shifted form
includes the -ln2, so no downstream corrections are needed.

Per-core pipeline (feature-major <-> item-major alternating, transpose-free):
  mm1 (W1 stationary, N=512)          -> z1[f2, t] PSUM
  Exp(z1 + b1) -> u1; Ln(.5u1+.5)     -> h[f2, t] bf16 (exact ssp)
  mm2 (h-tile stationary)             -> z2[t, f3] PSUM  (+rank-1 b2 matmul
                                         only when b2 != 0)
  Exp(z2) -> u2; Ln(.5u2+.5)          -> w[t, f3] bf16 (exact ssp)
  seg1 one-hot matmul                 -> w_ij[e', f] PSUM
  Win matmul (x_j pre-gathered)       -> f_j[e', f] PSUM
  DVE: wf = w_ij * f_j (bf16)
  seg2 one-hot matmul (16 acc)        -> conv[a', f] PSUM
  fp32 atom stage: PE transpose, Wout, exact ssp(+bout), Wd, +bd; y = v + x_a
"""
import sys

sys.path.insert(0, "/opt/trn_rl_repo")

import numpy as np
import ml_dtypes

LN2 = float(np.log(2.0))
N_ATOMS = 20000
DIM = 128
N_CORES = 8
ATOMS_PER_CORE = N_ATOMS // N_CORES
K2 = 16              # slot-tiles per atom window
BLK_TRIPLES = 256    # triple slots per 128-slot block

bf16 = ml_dtypes.bfloat16


# --------------------------------------------------------------------------
# Host-side packing
# --------------------------------------------------------------------------

def _pack_core(seg_i, seg_j, a0, a1):
    e0, e1 = np.searchsorted(seg_i, [a0, a1])
    t0, t1 = np.searchsorted(seg_j, [e0, e1])
    ne = np.diff(np.searchsorted(seg_j[t0:t1], np.arange(e0, e1 + 1))).astype(np.int64)
    atom_of_e = (seg_i[e0:e1] - a0).astype(np.int64)
    e_start_of_atom = np.searchsorted(seg_i[e0:e1] - a0, np.arange(a1 - a0 + 1))

    slot_e = []
    win_atoms = [[]]
    awin = []
    cur_slot = 0
    cur_trip = 0
    blk_in_win = 0
    atoms_in_win = 0
    cur_win = 0

    def close_block():
        nonlocal cur_slot, cur_trip, blk_in_win
        slot_e.extend([-1] * (128 - cur_slot))
        cur_slot = 0
        cur_trip = 0
        blk_in_win += 1

    def close_window():
        nonlocal blk_in_win, atoms_in_win, cur_win
        if cur_slot > 0 or cur_trip > 0:
            close_block()
        while blk_in_win < K2:
            slot_e.extend([-1] * 128)
            blk_in_win += 1
        blk_in_win = 0
        atoms_in_win = 0
        cur_win += 1
        win_atoms.append([])

    for a in range(a1 - a0):
        es, ee = int(e_start_of_atom[a]), int(e_start_of_atom[a + 1])
        if atoms_in_win >= 128:
            close_window()
        s_slot, s_trip, s_blk = cur_slot, cur_trip, blk_in_win
        ok = True
        for e in range(es, ee):
            t = int(ne[e])
            assert t <= BLK_TRIPLES
            if s_slot >= 128 or s_trip + t > BLK_TRIPLES:
                s_blk += 1
                s_slot = 0
                s_trip = 0
                if s_blk >= K2:
                    ok = False
                    break
            s_slot += 1
            s_trip += t
        if not ok:
            close_window()
        for e in range(es, ee):
            t = int(ne[e])
            if cur_slot >= 128 or cur_trip + t > BLK_TRIPLES:
                close_block()
                assert blk_in_win < K2
            slot_e.append(e + e0)
            cur_slot += 1
            cur_trip += t
        win_atoms[cur_win].append(a)
        awin.append((cur_win, len(win_atoms[cur_win]) - 1))
        atoms_in_win += 1
    close_window()
    win_atoms.pop()

    slot_e = np.asarray(slot_e, dtype=np.int64)
    return dict(e0=e0, e1=e1, ne=ne, atom_of_e=atom_of_e,
                slot_e=slot_e, awin=np.asarray(awin).reshape(-1, 2),
                n_win=len(win_atoms))


def _build_core_arrays(core, x, dijk, idx_j, seg_j, pk, B1, NW):
    ES = B1 * 128
    T1S = B1 * 256
    A_SLOTS = NW * 128
    e0 = pk["e0"]
    ne = pk["ne"]
    slot_e = pk["slot_e"]
    slot_e_pad = np.full(ES, -1, dtype=np.int64)
    slot_e_pad[:len(slot_e)] = slot_e
    valid = slot_e_pad >= 0

    tstart = np.searchsorted(seg_j, np.arange(pk["e0"], pk["e1"]))
    trip_src = np.full(T1S, -1, dtype=np.int64)
    m1_rows = np.full(T1S, -1, dtype=np.int64)
    sb = slot_e_pad.reshape(B1, 128)
    for b in range(B1):
        pos = 0
        base = b * 256
        for sp in range(128):
            e = sb[b, sp]
            if e < 0:
                continue
            k = int(ne[e - e0])
            if k == 0:
                continue
            ts_ = int(tstart[e - e0])
            trip_src[base + pos: base + pos + k] = np.arange(ts_, ts_ + k)
            m1_rows[base + pos: base + pos + k] = sp
            pos += k
        assert pos <= 256

    dT = np.zeros((DIM, T1S), dtype=bf16)
    m = trip_src >= 0
    dT[:, m] = dijk[trip_src[m]].T.astype(bf16)

    m1 = np.zeros((128, T1S), dtype=bf16)
    tj = np.arange(T1S) // 128
    tr = np.arange(T1S) % 128
    mm = m1_rows >= 0
    m1[tr[mm], tj[mm] * 128 + m1_rows[mm]] = 1.0

    xT = np.zeros((DIM, ES), dtype=bf16)
    xT[:, valid] = x[idx_j[slot_e_pad[valid]]].T.astype(bf16)

    m2 = np.zeros((128, ES), dtype=bf16)
    aw = pk["awin"]
    atom_loc = pk["atom_of_e"]
    sv = np.nonzero(valid)[0]
    e_loc = slot_e_pad[sv] - e0
    apos = aw[atom_loc[e_loc], 1]
    m2[sv % 128, (sv // 128) * 128 + apos] = 1.0

    xaT = np.zeros((DIM, A_SLOTS), dtype=np.float32)
    aslot_of_atom = aw[:, 0] * 128 + aw[:, 1]
    a0 = core * ATOMS_PER_CORE
    xaT[:, aslot_of_atom] = x[a0: a0 + ATOMS_PER_CORE].T
    return dict(dT=dT, m1=m1, xT=xT, m2=m2, xaT=xaT,
                aslot_of_atom=aslot_of_atom)


def _prepare(inputs):
    x = np.asarray(inputs["x"], dtype=np.float32)
    dijk = np.asarray(inputs["dijk"], dtype=np.float32)
    idx_j = np.asarray(inputs["idx_j"]).astype(np.int64)
    seg_i = np.asarray(inputs["seg_i"]).astype(np.int64)
    seg_j = np.asarray(inputs["seg_j"]).astype(np.int64)
    assert int(inputs["seg_i_sum"]) == N_ATOMS

    packs = [_pack_core(seg_i, seg_j, c * ATOMS_PER_CORE, (c + 1) * ATOMS_PER_CORE)
             for c in range(N_CORES)]
    NW = max(p["n_win"] for p in packs)
    B1 = NW * K2
    cores = [_build_core_arrays(c, x, dijk, idx_j, seg_j, packs[c], B1, NW)
             for c in range(N_CORES)]

    b2 = np.asarray(inputs["b2"], np.float32)
    use_b2 = bool(np.any(b2 != 0.0))
    b2hi = b2.astype(bf16)
    b2lo = (b2 - b2hi.astype(np.float32)).astype(bf16)
    consts = dict(
        W1=np.asarray(inputs["W1"], np.float32).astype(bf16),
        W2=np.asarray(inputs["W2"], np.float32).astype(bf16),
        Win=np.asarray(inputs["Win"], np.float32).astype(bf16),
        Wout=np.asarray(inputs["Wout"], np.float32),
        Wd=np.asarray(inputs["Wd"], np.float32),
        b1=np.asarray(inputs["b1"], np.float32).reshape(128, 1),
        bout=np.asarray(inputs["bout"], np.float32).reshape(128, 1),
        bd=np.asarray(inputs["bd"], np.float32).reshape(128, 1),
        bhilo=np.stack([b2hi, b2lo]).astype(bf16),
    )
    return dict(B1=B1, NW=NW, cores=cores, consts=consts, use_b2=use_b2)


# --------------------------------------------------------------------------
# Device program
# --------------------------------------------------------------------------

_PROGRAM_CACHE = {}


def _build_program(B1, NW, use_b2):
    import bass_rust as _bass_rust
    import concourse.mybir as mybir
    import concourse.tile as tile
    from concourse import bacc
    from concourse.hw_specs import get_activation_tables
    from concourse.masks import make_identity

    class _Bacc(bacc.Bacc):
        """Force every activation onto the single exp+ln table set so the
        Exp/Ln alternation never reloads ACT tables (index into the table
        list is the act_func_set_id, so other entries are emptied, not
        removed)."""

        def insert_act_table_loads(self):
            import concourse.mybir as mb
            has_activation = any(
                isinstance(i, mb.InstActivation)
                for b in self.main_func.blocks
                for i in b.instructions
            )
            if not has_activation:
                return
            tables = []
            for name, funcs in get_activation_tables(self.m.arch).items():
                if name == "natural_log_exp_and_others":
                    tables.append((name, funcs))
                else:
                    tables.append((name, type(funcs)()))
            _bass_rust.insert_act_table_loads(self, tables)

    dt = mybir.dt
    AF = mybir.ActivationFunctionType
    ES = B1 * 128
    T1S = B1 * 256
    A_SLOTS = NW * 128
    NI = B1 // 2          # pipeline iterations: 512 triple-slots (2 blocks)
    NS = NI // 4          # 2048-col slabs

    nc = _Bacc("TRN2", target_bir_lowering=False, debug=False)

    def din(name, shape, dtype):
        return nc.dram_tensor(name, shape, dtype, kind="ExternalInput").ap()

    dT_d = din("dT", [DIM, T1S], dt.bfloat16)
    m1_d = din("m1", [128, T1S], dt.bfloat16)
    xT_d = din("xT", [DIM, ES], dt.bfloat16)
    m2_d = din("m2", [128, ES], dt.bfloat16)
    xaT_d = din("xaT", [DIM, A_SLOTS], dt.float32)
    W1_d = din("W1", [128, 128], dt.bfloat16)
    W2_d = din("W2", [128, 128], dt.bfloat16)
    Win_d = din("Win", [128, 128], dt.bfloat16)
    Wout_d = din("Wout", [128, 128], dt.float32)
    Wd_d = din("Wd", [128, 128], dt.float32)
    b1_d = din("b1", [128, 1], dt.float32)
    bout_d = din("bout", [128, 1], dt.float32)
    bd_d = din("bd", [128, 1], dt.float32)
    bhilo_d = din("bhilo", [2, 128], dt.bfloat16)

    yT_d = nc.dram_tensor("yT", [DIM, A_SLOTS], dt.float32, kind="ExternalOutput").ap()
    vT_d = nc.dram_tensor("vT", [DIM, A_SLOTS], dt.float32, kind="ExternalOutput").ap()

    with tile.TileContext(nc) as tc:
        with (
            tc.tile_pool(name="const", bufs=1) as cpool,
            tc.tile_pool(name="dTp", bufs=3) as dT_pool,
            tc.tile_pool(name="m1p", bufs=7) as m1_pool,
            tc.tile_pool(name="xTp", bufs=7) as xT_pool,
            tc.tile_pool(name="m2p", bufs=8) as m2_pool,
            tc.tile_pool(name="u1p", bufs=2) as u1_pool,
            tc.tile_pool(name="u2p", bufs=2) as u2_pool,
            tc.tile_pool(name="hp", bufs=2) as h_pool,
            tc.tile_pool(name="wp", bufs=2) as w_pool,
            tc.tile_pool(name="fjp", bufs=4) as fj_pool,
            tc.tile_pool(name="wfp", bufs=12) as wf_pool,
            tc.tile_pool(name="atoms", bufs=2) as at_pool,
            tc.tile_pool(name="vy", bufs=3) as vy_pool,
            tc.tile_pool(name="zp", bufs=2, space="PSUM") as z_pool,
            tc.tile_pool(name="sfp", bufs=2, space="PSUM") as sf_pool,
            tc.tile_pool(name="atp", bufs=2, space="PSUM") as atp_pool,
        ):
            # ---- constants
            def cload(name, shape, dtype, src):
                t = cpool.tile(shape, dtype, tag=name, name=name)
                nc.sync.dma_start(out=t[:], in_=src[:])
                return t

            W1s = cload("W1s", [128, 128], dt.bfloat16, W1_d)
            W2s = cload("W2s", [128, 128], dt.bfloat16, W2_d)
            Wins = cload("Wins", [128, 128], dt.bfloat16, Win_d)
            Wouts = cload("Wouts", [128, 128], dt.float32, Wout_d)
            Wds = cload("Wds", [128, 128], dt.float32, Wd_d)
            b1s = cload("b1s", [128, 1], dt.float32, b1_d)
            bouts = cload("bouts", [128, 1], dt.float32, bout_d)
            bds = cload("bds", [128, 1], dt.float32, bd_d)
            bhilos = cload("bhilos", [2, 128], dt.bfloat16, bhilo_d)
            xaTs = cpool.tile([128, A_SLOTS], dt.float32, tag="xaTs", name="xaTs")
            ones2 = cpool.tile([2, 128], dt.bfloat16, tag="ones2")
            nc.vector.memset(ones2[:], 1.0)
            halfs = cpool.tile([128, 1], dt.float32, tag="halfs")
            nc.vector.memset(halfs[:], 0.5)
            ident = cpool.tile([128, 128], dt.float32, tag="ident")
            make_identity(nc, ident[:])

            NP = B1 // 4   # pipeline unit p: 1024 triple cols / 512 slots
            st = {}        # slab q (2048 cols) -> loaded tiles
            slabs = {}     # q -> u1/h/u2/w slabs
            st_m2 = {}     # q -> m2 slab
            wf_ring = []   # (block-pair index, tile[128,256])

            def ld(q):
                if not (0 <= q < NS):
                    return
                s = {}
                s["dT"] = dT_pool.tile([128, 2048], dt.bfloat16, tag="dT", name="dT")
                nc.sync.dma_start(out=s["dT"][:], in_=dT_d[:, q * 2048:(q + 1) * 2048])
                s["m1"] = m1_pool.tile([128, 2048], dt.bfloat16, tag="m1", name="m1")
                nc.sync.dma_start(out=s["m1"][:], in_=m1_d[:, q * 2048:(q + 1) * 2048])
                s["xT"] = xT_pool.tile([128, 1024], dt.bfloat16, tag="xT", name="xT")
                nc.sync.dma_start(out=s["xT"][:], in_=xT_d[:, q * 1024:(q + 1) * 1024])
                m2t = m2_pool.tile([128, 1024], dt.bfloat16, tag="m2", name="m2")
                nc.sync.dma_start(out=m2t[:], in_=m2_d[:, q * 1024:(q + 1) * 1024])
                st_m2[q] = m2t
                st[q] = s

            def s_mm1(p):
                # 1024 triple cols: 2 matmuls (N=512) + one Exp @1024
                if not (0 <= p < NP):
                    return
                q, rp = divmod(p, 2)
                if rp == 0:
                    slabs[q] = {"u1": u1_pool.tile([128, 2048], dt.float32,
                                                   tag="u1", name="u1")}
                z = z_pool.tile([128, 1024], dt.float32, tag="z", name="z")
                for half in range(2):
                    nc.tensor.matmul(
                        z[:, half * 512:(half + 1) * 512], lhsT=W1s[:],
                        rhs=st[q]["dT"][:, (rp * 2 + half) * 512:
                                        (rp * 2 + half + 1) * 512],
                        start=True, stop=True)
                nc.scalar.activation(slabs[q]["u1"][:, rp * 1024:(rp + 1) * 1024],
                                     z[:], AF.Exp, bias=b1s[:], scale=1.0)

            def s_ln1(q):
                if not (0 <= q < NS):
                    return
                h = h_pool.tile([128, 2048], dt.bfloat16, tag="h")
                nc.scalar.activation(h[:], slabs[q]["u1"][:], AF.Ln,
                                     bias=halfs[:], scale=0.5)
                slabs[q]["h"] = h

            def s_mm2(p):
                if not (0 <= p < NP):
                    return
                q, rp = divmod(p, 2)
                if rp == 0:
                    slabs[q]["u2"] = u2_pool.tile([128, 2048], dt.float32,
                                                  tag="u2", name="u2")
                h = slabs[q]["h"]
                z = z_pool.tile([128, 1024], dt.float32, tag="z", name="z2")
                for k in range(8):
                    sl = z[:, k * 128:(k + 1) * 128]
                    hsl = h[:, (rp * 8 + k) * 128:(rp * 8 + k + 1) * 128]
                    if use_b2:
                        nc.tensor.matmul(sl, lhsT=ones2[:], rhs=bhilos[:],
                                         start=True, stop=False)
                        nc.tensor.matmul(sl, lhsT=hsl, rhs=W2s[:],
                                         start=False, stop=True)
                    else:
                        nc.tensor.matmul(sl, lhsT=hsl, rhs=W2s[:],
                                         start=True, stop=True)
                nc.scalar.activation(slabs[q]["u2"][:, rp * 1024:(rp + 1) * 1024],
                                     z[:], AF.Exp)

            def s_ln2(q):
                if not (0 <= q < NS):
                    return
                w = w_pool.tile([128, 2048], dt.bfloat16, tag="w")
                nc.scalar.activation(w[:], slabs[q]["u2"][:], AF.Ln,
                                     bias=halfs[:], scale=0.5)
                slabs[q]["w"] = w

            def s_seg(p):
                # two sf tiles (2 blocks each)
                if not (0 <= p < NP):
                    return
                q, rp = divmod(p, 2)
                s = st[q]
                w = slabs[q]["w"]
                for half in range(2):
                    sf = sf_pool.tile([128, 512], dt.float32, tag="sf", name="sf")
                    for bb in range(2):
                        dst = sf[:, bb * 128:(bb + 1) * 128]
                        for k in range(2):
                            j = rp * 8 + half * 4 + 2 * bb + k
                            nc.tensor.matmul(
                                dst,
                                lhsT=s["m1"][:, j * 128:(j + 1) * 128],
                                rhs=w[:, j * 128:(j + 1) * 128],
                                start=(k == 0), stop=(k == 1))
                        xcol = rp * 4 + half * 2 + bb
                        nc.tensor.matmul(
                            sf[:, 256 + bb * 128:256 + (bb + 1) * 128],
                            lhsT=s["xT"][:, xcol * 128:(xcol + 1) * 128],
                            rhs=Wins[:], start=True, stop=True)
                    fj = fj_pool.tile([128, 256], dt.bfloat16, tag="fj")
                    nc.vector.tensor_copy(fj[:], sf[:, 256:512])
                    wf = wf_pool.tile([128, 256], dt.bfloat16, tag="wf")
                    nc.vector.tensor_mul(wf[:], sf[:, 0:256], fj[:])
                    wf_ring.append((2 * p + half, wf))

            def s_atom(wi):
                if not (0 <= wi < NW):
                    return
                at = atp_pool.tile([128, 512], dt.float32, tag="at", name="at")
                conv = at[:, 0:128]
                for k in range(K2):
                    b = wi * K2 + k
                    i2w, wf = wf_ring[0]
                    assert i2w == b // 2
                    m2t = st_m2[b // 8]
                    nc.tensor.matmul(conv,
                                     lhsT=m2t[:, (b % 8) * 128:(b % 8 + 1) * 128],
                                     rhs=wf[:, (b % 2) * 128:(b % 2 + 1) * 128],
                                     start=(k == 0), stop=(k == K2 - 1))
                    if b % 2 == 1:
                        wf_ring.pop(0)
                conv_s = at_pool.tile([128, 128], dt.float32, tag="conv_s")
                nc.vector.tensor_copy(conv_s[:], conv)
                nc.tensor.transpose(at[:, 128:256], conv_s[:], ident[:])
                cT = at_pool.tile([128, 128], dt.float32, tag="cT")
                nc.vector.tensor_copy(cT[:], at[:, 128:256])
                nc.tensor.matmul(at[:, 256:384], lhsT=Wouts[:], rhs=cT[:],
                                 start=True, stop=True)
                u3 = at_pool.tile([128, 128], dt.float32, tag="u3")
                nc.scalar.activation(u3[:], at[:, 256:384], AF.Exp, bias=bouts[:])
                c3 = at_pool.tile([128, 128], dt.float32, tag="c3")
                nc.scalar.activation(c3[:], u3[:], AF.Ln, bias=halfs[:], scale=0.5)
                nc.tensor.matmul(at[:, 384:512], lhsT=Wds[:], rhs=c3[:],
                                 start=True, stop=True)
                vt = vy_pool.tile([128, 128], dt.float32, tag="vt")
                nc.vector.tensor_scalar_add(vt[:], at[:, 384:512], bds[:])
                yt = vy_pool.tile([128, 128], dt.float32, tag="yt")
                nc.vector.tensor_add(yt[:], vt[:], xaTs[:, wi * 128:(wi + 1) * 128])
                nc.sync.dma_start(out=vT_d[:, wi * 128:(wi + 1) * 128], in_=vt[:])
                nc.sync.dma_start(out=yT_d[:, wi * 128:(wi + 1) * 128], in_=yt[:])

            # pipeline: ld | mm1(n-2) | ln1 | mm2(n-4) | ln2 | seg(n-6) | atom
            for n in range(NP + 9):
                if n == 0:
                    ld(0)
                if n % 2 == 0:
                    ld(n // 2 + 1)
                if n == 2:
                    # deferred so the first dT/m1 loads aren't queued behind it
                    nc.sync.dma_start(out=xaTs[:], in_=xaT_d[:])
                s_mm1(n - 2)
                if (n - 2) % 2 == 1:
                    s_ln1((n - 2) // 2)
                s_mm2(n - 4)
                if (n - 4) % 2 == 1:
                    s_ln2((n - 4) // 2)
                s_seg(n - 6)
                ps = n - 6
                if ps >= 0 and ps % 4 == 3:
                    s_atom(ps // 4)
    nc.compile()
    return nc


def _get_program(B1, NW, use_b2):
    key = (B1, NW, use_b2)
    if key not in _PROGRAM_CACHE:
        _PROGRAM_CACHE[key] = _build_program(B1, NW, use_b2)
    return _PROGRAM_CACHE[key]


def kernel(**inputs):
    from concourse.bass_utils import run_bass_kernel_spmd

    prep = _prepare(inputs)
    nc = _get_program(prep["B1"], prep["NW"], prep["use_b2"])
    consts = prep["consts"]
    in_maps = []
    for c in range(N_CORES):
        cc = prep["cores"][c]
        in_maps.append(dict(
            dT=np.ascontiguousarray(cc["dT"]),
            m1=np.ascontiguousarray(cc["m1"]),
            xT=np.ascontiguousarray(cc["xT"]),
            m2=np.ascontiguousarray(cc["m2"]),
            xaT=np.ascontiguousarray(cc["xaT"]),
            W1=consts["W1"], W2=consts["W2"], Win=consts["Win"],
            Wout=consts["Wout"], Wd=consts["Wd"],
            b1=consts["b1"], bout=consts["bout"], bd=consts["bd"],
            bhilo=consts["bhilo"],
        ))
    res = run_bass_kernel_spmd(nc, in_maps, list(range(N_CORES)))
    y = np.zeros((N_ATOMS, DIM), np.float32)
    v = np.zeros((N_ATOMS, DIM), np.float32)
    for c in range(N_CORES):
        sl = prep["cores"][c]["aslot_of_atom"]
        a0 = c * ATOMS_PER_CORE
        y[a0:a0 + ATOMS_PER_CORE] = res.results[c]["yT"][:, sl].T
        v[a0:a0 + ATOMS_PER_CORE] = res.results[c]["vT"][:, sl].T
    return y, v


# revision 16
# speedup vs baseline: 250.4711x; 1.0468x over previous
"""CFNet interaction block on 8 TRN2 NeuronCores (Bass/Tile).

Sharding: core c owns atoms [2500c, 2500(c+1)). seg_i/seg_j sorted => each
core's interactions and triples are contiguous ranges; all index prep is done
host-side in numpy, so no device collectives or gathers are needed.

Both segment-sums are expressed as one-hot matmuls on the tensor engine:
interactions are packed into 128-slot blocks (<=256 triples each, 2 triple
tiles per block); K2=16 slot-tiles form an atom window (<=128 atoms) for the
atom-dim segment sum.

ssp(z) = softplus(z) - ln2 is computed exactly in 2 ACT passes (no softplus
table in this toolchain): u = Exp(z); ssp = Ln(0.5*u + 0.5). Both live in the
natural_log_exp_and_others table set (single table load). The shifted form
includes the -ln2, so no downstream corrections are needed.

Per-core pipeline (feature-major <-> item-major alternating, transpose-free):
  mm1 (W1 stationary, N=512)          -> z1[f2, t] PSUM
  Exp(z1 + b1) -> u1; Ln(.5u1+.5)     -> h[f2, t] bf16 (exact ssp)
  mm2 (h-tile stationary)             -> z2[t, f3] PSUM  (+rank-1 b2 matmul
                                         only when b2 != 0)
  Exp(z2) -> u2; Ln(.5u2+.5)          -> w[t, f3] bf16 (exact ssp)
  seg1 one-hot matmul                 -> w_ij[e', f] PSUM
  Win matmul (x_j pre-gathered)       -> f_j[e', f] PSUM
  DVE: wf = w_ij * f_j (bf16)
  seg2 one-hot matmul (16 acc)        -> conv[a', f] PSUM
  fp32 atom stage: PE transpose, Wout, exact ssp(+bout), Wd, +bd; y = v + x_a
"""
import sys

sys.path.insert(0, "/opt/trn_rl_repo")

import numpy as np
import ml_dtypes

LN2 = float(np.log(2.0))
N_ATOMS = 20000
DIM = 128
N_CORES = 8
ATOMS_PER_CORE = N_ATOMS // N_CORES
K2 = 16              # slot-tiles per atom window
BLK_TRIPLES = 256    # triple slots per 128-slot block

bf16 = ml_dtypes.bfloat16


# --------------------------------------------------------------------------
# Host-side packing
# --------------------------------------------------------------------------

def _pack_core(seg_i, seg_j, a0, a1):
    e0, e1 = np.searchsorted(seg_i, [a0, a1])
    t0, t1 = np.searchsorted(seg_j, [e0, e1])
    ne = np.diff(np.searchsorted(seg_j[t0:t1], np.arange(e0, e1 + 1))).astype(np.int64)
    atom_of_e = (seg_i[e0:e1] - a0).astype(np.int64)
    e_start_of_atom = np.searchsorted(seg_i[e0:e1] - a0, np.arange(a1 - a0 + 1))

    slot_e = []
    win_atoms = [[]]
    awin = []
    cur_slot = 0
    cur_trip = 0
    blk_in_win = 0
    atoms_in_win = 0
    cur_win = 0

    def close_block():
        nonlocal cur_slot, cur_trip, blk_in_win
        slot_e.extend([-1] * (128 - cur_slot))
        cur_slot = 0
        cur_trip = 0
        blk_in_win += 1

    def close_window():
        nonlocal blk_in_win, atoms_in_win, cur_win
        if cur_slot > 0 or cur_trip > 0:
            close_block()
        while blk_in_win < K2:
            slot_e.extend([-1] * 128)
            blk_in_win += 1
        blk_in_win = 0
        atoms_in_win = 0
        cur_win += 1
        win_atoms.append([])

    for a in range(a1 - a0):
        es, ee = int(e_start_of_atom[a]), int(e_start_of_atom[a + 1])
        if atoms_in_win >= 128:
            close_window()
        s_slot, s_trip, s_blk = cur_slot, cur_trip, blk_in_win
        ok = True
        for e in range(es, ee):
            t = int(ne[e])
            assert t <= BLK_TRIPLES
            if t == 0:
                continue  # empty interaction: w_ij = 0, needs no slot
            if s_slot >= 128 or s_trip + t > BLK_TRIPLES:
                s_blk += 1
                s_slot = 0
                s_trip = 0
                if s_blk >= K2:
                    ok = False
                    break
            s_slot += 1
            s_trip += t
        if not ok:
            close_window()
        for e in range(es, ee):
            t = int(ne[e])
            if t == 0:
                continue
            if cur_slot >= 128 or cur_trip + t > BLK_TRIPLES:
                close_block()
                assert blk_in_win < K2
            slot_e.append(e + e0)
            cur_slot += 1
            cur_trip += t
        win_atoms[cur_win].append(a)
        awin.append((cur_win, len(win_atoms[cur_win]) - 1))
        atoms_in_win += 1
    close_window()
    win_atoms.pop()

    slot_e = np.asarray(slot_e, dtype=np.int64)
    return dict(e0=e0, e1=e1, ne=ne, atom_of_e=atom_of_e,
                slot_e=slot_e, awin=np.asarray(awin).reshape(-1, 2),
                n_win=len(win_atoms))


def _build_core_arrays(core, x, dijk, idx_j, seg_j, pk, B1, NW):
    ES = B1 * 128
    T1S = B1 * 256
    A_SLOTS = NW * 128
    e0 = pk["e0"]
    ne = pk["ne"]
    slot_e = pk["slot_e"]
    slot_e_pad = np.full(ES, -1, dtype=np.int64)
    slot_e_pad[:len(slot_e)] = slot_e
    valid = slot_e_pad >= 0

    tstart = np.searchsorted(seg_j, np.arange(pk["e0"], pk["e1"]))
    trip_src = np.full(T1S, -1, dtype=np.int64)
    m1_rows = np.full(T1S, -1, dtype=np.int64)
    sb = slot_e_pad.reshape(B1, 128)
    for b in range(B1):
        pos = 0
        base = b * 256
        for sp in range(128):
            e = sb[b, sp]
            if e < 0:
                continue
            k = int(ne[e - e0])
            if k == 0:
                continue
            ts_ = int(tstart[e - e0])
            trip_src[base + pos: base + pos + k] = np.arange(ts_, ts_ + k)
            m1_rows[base + pos: base + pos + k] = sp
            pos += k
        assert pos <= 256

    dT = np.zeros((DIM, T1S), dtype=bf16)
    m = trip_src >= 0
    dT[:, m] = dijk[trip_src[m]].T.astype(bf16)

    m1 = np.zeros((128, T1S), dtype=bf16)
    tj = np.arange(T1S) // 128
    tr = np.arange(T1S) % 128
    mm = m1_rows >= 0
    m1[tr[mm], tj[mm] * 128 + m1_rows[mm]] = 1.0

    xT = np.zeros((DIM, ES), dtype=bf16)
    xT[:, valid] = x[idx_j[slot_e_pad[valid]]].T.astype(bf16)

    m2 = np.zeros((128, ES), dtype=bf16)
    aw = pk["awin"]
    atom_loc = pk["atom_of_e"]
    sv = np.nonzero(valid)[0]
    e_loc = slot_e_pad[sv] - e0
    apos = aw[atom_loc[e_loc], 1]
    m2[sv % 128, (sv // 128) * 128 + apos] = 1.0

    xaT = np.zeros((DIM, A_SLOTS), dtype=np.float32)
    aslot_of_atom = aw[:, 0] * 128 + aw[:, 1]
    a0 = core * ATOMS_PER_CORE
    xaT[:, aslot_of_atom] = x[a0: a0 + ATOMS_PER_CORE].T
    return dict(dT=dT, m1=m1, xT=xT, m2=m2, xaT=xaT,
                aslot_of_atom=aslot_of_atom)


def _prepare(inputs):
    x = np.asarray(inputs["x"], dtype=np.float32)
    dijk = np.asarray(inputs["dijk"], dtype=np.float32)
    idx_j = np.asarray(inputs["idx_j"]).astype(np.int64)
    seg_i = np.asarray(inputs["seg_i"]).astype(np.int64)
    seg_j = np.asarray(inputs["seg_j"]).astype(np.int64)
    assert int(inputs["seg_i_sum"]) == N_ATOMS

    packs = [_pack_core(seg_i, seg_j, c * ATOMS_PER_CORE, (c + 1) * ATOMS_PER_CORE)
             for c in range(N_CORES)]
    NW = max(p["n_win"] for p in packs)
    B1 = NW * K2
    cores = [_build_core_arrays(c, x, dijk, idx_j, seg_j, packs[c], B1, NW)
             for c in range(N_CORES)]

    b2 = np.asarray(inputs["b2"], np.float32)
    use_b2 = bool(np.any(b2 != 0.0))
    b2hi = b2.astype(bf16)
    b2lo = (b2 - b2hi.astype(np.float32)).astype(bf16)
    consts = dict(
        W1=np.asarray(inputs["W1"], np.float32).astype(bf16),
        W2=np.asarray(inputs["W2"], np.float32).astype(bf16),
        Win=np.asarray(inputs["Win"], np.float32).astype(bf16),
        Wout=np.asarray(inputs["Wout"], np.float32),
        Wd=np.asarray(inputs["Wd"], np.float32),
        b1=np.asarray(inputs["b1"], np.float32).reshape(128, 1),
        bout=np.asarray(inputs["bout"], np.float32).reshape(128, 1),
        bd=np.asarray(inputs["bd"], np.float32).reshape(128, 1),
        bhilo=np.stack([b2hi, b2lo]).astype(bf16),
    )
    return dict(B1=B1, NW=NW, cores=cores, consts=consts, use_b2=use_b2)


# --------------------------------------------------------------------------
# Device program
# --------------------------------------------------------------------------

_PROGRAM_CACHE = {}


def _build_program(B1, NW, use_b2):
    import bass_rust as _bass_rust
    import concourse.mybir as mybir
    import concourse.tile as tile
    from concourse import bacc
    from concourse.hw_specs import get_activation_tables
    from concourse.masks import make_identity

    class _Bacc(bacc.Bacc):
        """Force every activation onto the single exp+ln table set so the
        Exp/Ln alternation never reloads ACT tables (index into the table
        list is the act_func_set_id, so other entries are emptied, not
        removed)."""

        def insert_act_table_loads(self):
            import concourse.mybir as mb
            has_activation = any(
                isinstance(i, mb.InstActivation)
                for b in self.main_func.blocks
                for i in b.instructions
            )
            if not has_activation:
                return
            tables = []
            for name, funcs in get_activation_tables(self.m.arch).items():
                if name == "natural_log_exp_and_others":
                    tables.append((name, funcs))
                else:
                    tables.append((name, type(funcs)()))
            _bass_rust.insert_act_table_loads(self, tables)

    dt = mybir.dt
    AF = mybir.ActivationFunctionType
    ES = B1 * 128
    T1S = B1 * 256
    A_SLOTS = NW * 128
    NI = B1 // 2          # pipeline iterations: 512 triple-slots (2 blocks)
    NS = NI // 4          # 2048-col slabs

    nc = _Bacc("TRN2", target_bir_lowering=False, debug=False)

    def din(name, shape, dtype):
        return nc.dram_tensor(name, shape, dtype, kind="ExternalInput").ap()

    dT_d = din("dT", [DIM, T1S], dt.bfloat16)
    m1_d = din("m1", [128, T1S], dt.bfloat16)
    xT_d = din("xT", [DIM, ES], dt.bfloat16)
    m2_d = din("m2", [128, ES], dt.bfloat16)
    xaT_d = din("xaT", [DIM, A_SLOTS], dt.float32)
    W1_d = din("W1", [128, 128], dt.bfloat16)
    W2_d = din("W2", [128, 128], dt.bfloat16)
    Win_d = din("Win", [128, 128], dt.bfloat16)
    Wout_d = din("Wout", [128, 128], dt.float32)
    Wd_d = din("Wd", [128, 128], dt.float32)
    b1_d = din("b1", [128, 1], dt.float32)
    bout_d = din("bout", [128, 1], dt.float32)
    bd_d = din("bd", [128, 1], dt.float32)
    bhilo_d = din("bhilo", [2, 128], dt.bfloat16)

    yT_d = nc.dram_tensor("yT", [DIM, A_SLOTS], dt.float32, kind="ExternalOutput").ap()
    vT_d = nc.dram_tensor("vT", [DIM, A_SLOTS], dt.float32, kind="ExternalOutput").ap()

    with tile.TileContext(nc) as tc:
        with (
            tc.tile_pool(name="const", bufs=1) as cpool,
            tc.tile_pool(name="dTp", bufs=3) as dT_pool,
            tc.tile_pool(name="m1p", bufs=7) as m1_pool,
            tc.tile_pool(name="xTp", bufs=7) as xT_pool,
            tc.tile_pool(name="m2p", bufs=8) as m2_pool,
            tc.tile_pool(name="u1p", bufs=2) as u1_pool,
            tc.tile_pool(name="u2p", bufs=2) as u2_pool,
            tc.tile_pool(name="hp", bufs=2) as h_pool,
            tc.tile_pool(name="wp", bufs=2) as w_pool,
            tc.tile_pool(name="fjp", bufs=4) as fj_pool,
            tc.tile_pool(name="wfp", bufs=12) as wf_pool,
            tc.tile_pool(name="atoms", bufs=2) as at_pool,
            tc.tile_pool(name="vy", bufs=3) as vy_pool,
            tc.tile_pool(name="zp", bufs=2, space="PSUM") as z_pool,
            tc.tile_pool(name="sfp", bufs=2, space="PSUM") as sf_pool,
            tc.tile_pool(name="atp", bufs=2, space="PSUM") as atp_pool,
        ):
            # ---- constants
            def cload(name, shape, dtype, src):
                t = cpool.tile(shape, dtype, tag=name, name=name)
                nc.sync.dma_start(out=t[:], in_=src[:])
                return t

            W1s = cload("W1s", [128, 128], dt.bfloat16, W1_d)
            W2s = cload("W2s", [128, 128], dt.bfloat16, W2_d)
            Wins = cload("Wins", [128, 128], dt.bfloat16, Win_d)
            Wouts = cload("Wouts", [128, 128], dt.float32, Wout_d)
            Wds = cload("Wds", [128, 128], dt.float32, Wd_d)
            b1s = cload("b1s", [128, 1], dt.float32, b1_d)
            bouts = cload("bouts", [128, 1], dt.float32, bout_d)
            bds = cload("bds", [128, 1], dt.float32, bd_d)
            bhilos = cload("bhilos", [2, 128], dt.bfloat16, bhilo_d)
            xaTs = cpool.tile([128, A_SLOTS], dt.float32, tag="xaTs", name="xaTs")
            ones2 = cpool.tile([2, 128], dt.bfloat16, tag="ones2")
            nc.vector.memset(ones2[:], 1.0)
            halfs = cpool.tile([128, 1], dt.float32, tag="halfs")
            nc.vector.memset(halfs[:], 0.5)
            ident = cpool.tile([128, 128], dt.float32, tag="ident")
            make_identity(nc, ident[:])

            NP = B1 // 4   # pipeline unit p: 1024 triple cols / 512 slots
            NP = B1 // 4   # pipeline unit p: 1024 triple cols / 512 slots
            st = {}        # slab q (2048 cols) -> loaded tiles
            slabs = {}     # q -> u1/h/u2/w slabs
            st_m2 = {}     # q -> m2 slab
            wf_ring = []   # (block-pair index, tile[128,256])

            def ld(q):
                if not (0 <= q < NS):
                    return
                s = {}
                s["dT"] = dT_pool.tile([128, 2048], dt.bfloat16, tag="dT", name="dT")
                nc.sync.dma_start(out=s["dT"][:], in_=dT_d[:, q * 2048:(q + 1) * 2048])
                s["m1"] = m1_pool.tile([128, 2048], dt.bfloat16, tag="m1", name="m1")
                nc.sync.dma_start(out=s["m1"][:], in_=m1_d[:, q * 2048:(q + 1) * 2048])
                s["xT"] = xT_pool.tile([128, 1024], dt.bfloat16, tag="xT", name="xT")
                nc.sync.dma_start(out=s["xT"][:], in_=xT_d[:, q * 1024:(q + 1) * 1024])
                m2t = m2_pool.tile([128, 1024], dt.bfloat16, tag="m2", name="m2")
                nc.sync.dma_start(out=m2t[:], in_=m2_d[:, q * 1024:(q + 1) * 1024])
                st_m2[q] = m2t
                st[q] = s

            def s_mm1(p):
                # 1024 triple cols: 2 matmuls (N=512) + one Exp @1024
                if not (0 <= p < NP):
                    return
                q, rp = divmod(p, 2)
                if rp == 0:
                    slabs[q] = {"u1": u1_pool.tile([128, 2048], dt.float32,
                                                   tag="u1", name="u1")}
                z = z_pool.tile([128, 1024], dt.float32, tag="z", name="z")
                for half in range(2):
                    nc.tensor.matmul(
                        z[:, half * 512:(half + 1) * 512], lhsT=W1s[:],
                        rhs=st[q]["dT"][:, (rp * 2 + half) * 512:
                                        (rp * 2 + half + 1) * 512],
                        start=True, stop=True)
                nc.scalar.activation(slabs[q]["u1"][:, rp * 1024:(rp + 1) * 1024],
                                     z[:], AF.Exp, bias=b1s[:], scale=1.0)

            def s_ln1(q):
                if not (0 <= q < NS):
                    return
                h = h_pool.tile([128, 2048], dt.bfloat16, tag="h")
                nc.scalar.activation(h[:], slabs[q]["u1"][:], AF.Ln,
                                     bias=halfs[:], scale=0.5)
                slabs[q]["h"] = h

            def s_mm2(p):
                if not (0 <= p < NP):
                    return
                q, rp = divmod(p, 2)
                if rp == 0:
                    slabs[q]["u2"] = u2_pool.tile([128, 2048], dt.float32,
                                                  tag="u2", name="u2")
                h = slabs[q]["h"]
                z = z_pool.tile([128, 1024], dt.float32, tag="z", name="z2")
                for k in range(8):
                    sl = z[:, k * 128:(k + 1) * 128]
                    hsl = h[:, (rp * 8 + k) * 128:(rp * 8 + k + 1) * 128]
                    if use_b2:
                        nc.tensor.matmul(sl, lhsT=ones2[:], rhs=bhilos[:],
                                         start=True, stop=False)
                        nc.tensor.matmul(sl, lhsT=hsl, rhs=W2s[:],
                                         start=False, stop=True)
                    else:
                        nc.tensor.matmul(sl, lhsT=hsl, rhs=W2s[:],
                                         start=True, stop=True)
                nc.scalar.activation(slabs[q]["u2"][:, rp * 1024:(rp + 1) * 1024],
                                     z[:], AF.Exp)

            def s_ln2(q):
                if not (0 <= q < NS):
                    return
                w = w_pool.tile([128, 2048], dt.bfloat16, tag="w")
                nc.scalar.activation(w[:], slabs[q]["u2"][:], AF.Ln,
                                     bias=halfs[:], scale=0.5)
                slabs[q]["w"] = w

            def s_seg(p):
                # two sf tiles (2 blocks each)
                if not (0 <= p < NP):
                    return
                q, rp = divmod(p, 2)
                s = st[q]
                w = slabs[q]["w"]
                for half in range(2):
                    sf = sf_pool.tile([128, 512], dt.float32, tag="sf", name="sf")
                    for bb in range(2):
                        dst = sf[:, bb * 128:(bb + 1) * 128]
                        for k in range(2):
                            j = rp * 8 + half * 4 + 2 * bb + k
                            nc.tensor.matmul(
                                dst,
                                lhsT=s["m1"][:, j * 128:(j + 1) * 128],
                                rhs=w[:, j * 128:(j + 1) * 128],
                                start=(k == 0), stop=(k == 1))
                        xcol = rp * 4 + half * 2 + bb
                        nc.tensor.matmul(
                            sf[:, 256 + bb * 128:256 + (bb + 1) * 128],
                            lhsT=s["xT"][:, xcol * 128:(xcol + 1) * 128],
                            rhs=Wins[:], start=True, stop=True)
                    fj = fj_pool.tile([128, 256], dt.bfloat16, tag="fj")
                    nc.vector.tensor_copy(fj[:], sf[:, 256:512])
                    wf = wf_pool.tile([128, 256], dt.bfloat16, tag="wf")
                    nc.vector.tensor_mul(wf[:], sf[:, 0:256], fj[:])
                    wf_ring.append((2 * p + half, wf))

            def s_atom(wi):
                if not (0 <= wi < NW):
                    return
                at = atp_pool.tile([128, 512], dt.float32, tag="at", name="at")
                conv = at[:, 0:128]
                for k in range(K2):
                    b = wi * K2 + k
                    i2w, wf = wf_ring[0]
                    assert i2w == b // 2
                    m2t = st_m2[b // 8]
                    nc.tensor.matmul(conv,
                                     lhsT=m2t[:, (b % 8) * 128:(b % 8 + 1) * 128],
                                     rhs=wf[:, (b % 2) * 128:(b % 2 + 1) * 128],
                                     start=(k == 0), stop=(k == K2 - 1))
                    if b % 2 == 1:
                        wf_ring.pop(0)
                conv_s = at_pool.tile([128, 128], dt.float32, tag="conv_s")
                nc.vector.tensor_copy(conv_s[:], conv)
                nc.tensor.transpose(at[:, 128:256], conv_s[:], ident[:])
                cT = at_pool.tile([128, 128], dt.float32, tag="cT")
                nc.vector.tensor_copy(cT[:], at[:, 128:256])
                nc.tensor.matmul(at[:, 256:384], lhsT=Wouts[:], rhs=cT[:],
                                 start=True, stop=True)
                u3 = at_pool.tile([128, 128], dt.float32, tag="u3")
                nc.scalar.activation(u3[:], at[:, 256:384], AF.Exp, bias=bouts[:])
                c3 = at_pool.tile([128, 128], dt.float32, tag="c3")
                nc.scalar.activation(c3[:], u3[:], AF.Ln, bias=halfs[:], scale=0.5)
                nc.tensor.matmul(at[:, 384:512], lhsT=Wds[:], rhs=c3[:],
                                 start=True, stop=True)
                vt = vy_pool.tile([128, 128], dt.float32, tag="vt")
                nc.vector.tensor_scalar_add(vt[:], at[:, 384:512], bds[:])
                yt = vy_pool.tile([128, 128], dt.float32, tag="yt")
                nc.vector.tensor_add(yt[:], vt[:], xaTs[:, wi * 128:(wi + 1) * 128])
                nc.sync.dma_start(out=vT_d[:, wi * 128:(wi + 1) * 128], in_=vt[:])
                nc.sync.dma_start(out=yT_d[:, wi * 128:(wi + 1) * 128], in_=yt[:])

            # pipeline: ld | mm1(n-2) | ln1 | mm2(n-4) | ln2 | seg(n-6) | atom
            for n in range(NP + 9):
                if n == 0:
                    ld(0)
                if n % 2 == 0:
                    ld(n // 2 + 1)
                if n == 2:
                    # deferred so the first dT/m1 loads aren't queued behind it
                    nc.sync.dma_start(out=xaTs[:], in_=xaT_d[:])
                s_mm1(n - 2)
                if (n - 2) % 2 == 1:
                    s_ln1((n - 2) // 2)
                s_mm2(n - 4)
                if (n - 4) % 2 == 1:
                    s_ln2((n - 4) // 2)
                s_seg(n - 6)
                ps = n - 6
                if ps >= 0 and ps % 4 == 3:
                    s_atom(ps // 4)
    nc.compile()
    return nc


def _get_program(B1, NW, use_b2):
    key = (B1, NW, use_b2)
    if key not in _PROGRAM_CACHE:
        _PROGRAM_CACHE[key] = _build_program(B1, NW, use_b2)
    return _PROGRAM_CACHE[key]


def kernel(**inputs):
    from concourse.bass_utils import run_bass_kernel_spmd

    prep = _prepare(inputs)
    nc = _get_program(prep["B1"], prep["NW"], prep["use_b2"])
    consts = prep["consts"]
    in_maps = []
    for c in range(N_CORES):
        cc = prep["cores"][c]
        in_maps.append(dict(
            dT=np.ascontiguousarray(cc["dT"]),
            m1=np.ascontiguousarray(cc["m1"]),
            xT=np.ascontiguousarray(cc["xT"]),
            m2=np.ascontiguousarray(cc["m2"]),
            xaT=np.ascontiguousarray(cc["xaT"]),
            W1=consts["W1"], W2=consts["W2"], Win=consts["Win"],
            Wout=consts["Wout"], Wd=consts["Wd"],
            b1=consts["b1"], bout=consts["bout"], bd=consts["bd"],
            bhilo=consts["bhilo"],
        ))
    res = run_bass_kernel_spmd(nc, in_maps, list(range(N_CORES)))
    y = np.zeros((N_ATOMS, DIM), np.float32)
    v = np.zeros((N_ATOMS, DIM), np.float32)
    for c in range(N_CORES):
        sl = prep["cores"][c]["aslot_of_atom"]
        a0 = c * ATOMS_PER_CORE
        y[a0:a0 + ATOMS_PER_CORE] = res.results[c]["yT"][:, sl].T
        v[a0:a0 + ATOMS_PER_CORE] = res.results[c]["vT"][:, sl].T
    return y, v
